# revision 30
# baseline (speedup 1.0000x reference)
"""RNN-T decoder + joint + loss as a Bass/Tile kernel on 8 TRN2 NeuronCores.

Strategy:
  - LSTM decoder (2x512, seq 51) replicated on all cores, banded gate
    layout: gates (g,i,o,f) live in 4 partition bands at 0/32/64/96 so
    elementwise work is a handful of full-width DVE/ACT/Pool ops.
    The two layers are block-pipelined against each other so the
    per-step serial chain of one layer hides under the matmuls of the
    other.
  - Joint network sharded over T (25 frames per core): z1=tanh(enc+dec)
    in bf16, big GEMM vs lin_out, exp with accumulate for the softmax
    denominator, and a single fused scalar_tensor_tensor on GpSimd for
    the label-probability gather.
  - Blank/label probabilities (shifted by e^C) all-gathered in bf16,
    then every core runs the RNN-T alpha recursion redundantly in
    probability space using tensor_tensor_scan, renormalizing
    periodically.
"""
import os
import sys
sys.path.insert(0, '/opt/trn_rl_repo')
# A stale executable cache has been observed to return corrupted results for
# previously-compiled programs (NaN loss with normal exec time). Force fresh
# compilation; must be set before jax/axon initialize.
os.environ["JAX_ENABLE_COMPILATION_CACHE"] = "false"

import numpy as np
import ml_dtypes

import concourse.bacc as bacc
import concourse.mybir as mybir
from concourse import bass_utils
from concourse.tile import TileContext

try:  # harden against a pre-imported jax with caching already on
    import jax
    jax.config.update("jax_enable_compilation_cache", False)
except Exception:
    pass

F32 = mybir.dt.float32
BF16 = mybir.dt.bfloat16
BF = ml_dtypes.bfloat16
AF = mybir.ActivationFunctionType
OP = mybir.AluOpType

B, T, U = 8, 200, 50
U1 = U + 1            # 51
D = 512
G = 2048              # 4 * D gates
NCORES = 8
TSH = T // NCORES     # 25 t per core
NTB = TSH * B         # 200 (t,b) pairs per core
ROWS = NTB * U1       # 10200 joint rows per core
RPAD = 10240          # padded to 80 m-tiles of 128
NMT = RPAD // 128     # 80
C_SHIFT = float(np.log(512.0))
EC = 512.0            # e^C_SHIFT
RENORM_EVERY = 25
# gate band order on partitions 0/32/64/96: (g, i, o, f)
# PyTorch LSTM row order: i, f, g, o
PERM = np.concatenate([np.arange(1024, 1536),   # g
                       np.arange(0, 512),       # i
                       np.arange(1536, 2048),   # o
                       np.arange(512, 1024)])   # f
UBLOCKS = [(0, 8), (8, 16), (16, 24), (24, 32), (32, 40),
           (40, 48), (48, 51)]
SHIFT = 8             # layer-1 lags layer-0 by one block

_BUILD_CACHE = {}


def build():
    nc = bacc.Bacc("TRN2", target_bir_lowering=False, debug=False,
                   num_devices=NCORES)

    # ---------------- I/O ----------------
    eysT_d = nc.dram_tensor("eysT", [D, U1 * B], BF16, kind="ExternalInput")
    wih0T_d = nc.dram_tensor("wih0T", [D, G], BF16, kind="ExternalInput")
    whh0T_d = nc.dram_tensor("whh0T", [D, G], BF16, kind="ExternalInput")
    wih1T_d = nc.dram_tensor("wih1T", [D, G], BF16, kind="ExternalInput")
    whh1T_d = nc.dram_tensor("whh1T", [D, G], BF16, kind="ExternalInput")
    bias0_d = nc.dram_tensor("bias0", [1, G], BF16, kind="ExternalInput")
    bias1_d = nc.dram_tensor("bias1", [1, G], BF16, kind="ExternalInput")
    hsT_d = nc.dram_tensor("hsT", [D, NTB], BF16, kind="ExternalInput")
    lencT_d = nc.dram_tensor("lencT", [D, D], BF16, kind="ExternalInput")
    ldecT_d = nc.dram_tensor("ldecT", [D, D], BF16, kind="ExternalInput")
    loutT_d = nc.dram_tensor("loutT", [D, D], BF16, kind="ExternalInput")
    lencb_d = nc.dram_tensor("lencb", [128, 4], F32, kind="ExternalInput")
    loutb_d = nc.dram_tensor("loutb", [1, D], BF16, kind="ExternalInput")
    ysidx_d = nc.dram_tensor("ysidx", [128, NMT], F32, kind="ExternalInput")
    ident8_d = nc.dram_tensor("ident8", [8, 8], BF16, kind="ExternalInput")

    loss_d = nc.dram_tensor("loss", [1, 1], F32, kind="ExternalOutput")

    # internal DRAM
    xp_i = [nc.dram_tensor(f"xp{l}_i", [U1, 104, 512], F32, kind="Internal")
            for l in (0, 1)]
    lp_loc = nc.dram_tensor("lp_loc_r2", [2 * RPAD], BF16, kind="Internal")
    lp_full = nc.dram_tensor("lp_full_r2", [NCORES * 2 * RPAD], BF16,
                             kind="Internal", addr_space="Shared")

    with TileContext(nc) as tc:
        with tc.tile_pool(name="persist", bufs=1) as pp:
            # ---- persistent SBUF tensors ----
            # hT[l]: transposed hidden states, columns (k, u, b)
            hT = {l: pp.tile([128, 4 * U1 * 8], BF16, tag=f"hT{l}",
                             name=f"hT{l}") for l in (0, 1)}
            ldecT = []
            for k in range(4):
                t = pp.tile([128, D], BF16, tag=f"ldecT_{k}", name=f"ldecT_{k}")
                nc.sync.dma_start(t[:, :], ldecT_d.ap()[128 * k:128 * (k + 1), :])
                ldecT.append(t)
            loutT = []
            for k in range(4):
                t = pp.tile([128, D], BF16, tag=f"loutT_{k}", name=f"loutT_{k}")
                nc.sync.dma_start(t[:, :], loutT_d.ap()[128 * k:128 * (k + 1), :])
                loutT.append(t)
            lencb = pp.tile([128, 4], F32, tag="lencb", name="lencb")
            nc.sync.dma_start(lencb[:, :], lencb_d.ap())
            loutb = pp.tile([1, D], BF16, tag="loutb", name="loutb")
            nc.sync.dma_start(loutb[:, :], loutb_d.ap())
            ysidx = pp.tile([128, NMT], F32, tag="ysidx", name="ysidx")
            nc.sync.dma_start(ysidx[:, :], ysidx_d.ap())
            ident8 = pp.tile([8, 8], BF16, tag="ident8", name="ident8")
            nc.sync.dma_start(ident8[:, :], ident8_d.ap())
            onesb = pp.tile([1, 128], BF16, tag="onesb", name="onesb")
            nc.vector.memset(onesb[:, :], 1.0)
            iot = pp.tile([128, D], F32, tag="iot", name="iot")
            nc.gpsimd.iota(iot[:, :], pattern=[[1, D]], channel_multiplier=0,
                           allow_small_or_imprecise_dtypes=True)

            encT = [pp.tile([128, NTB], BF16, tag=f"encT_{k}", name=f"encT_{k}")
                    for k in range(4)]
            decT = [pp.tile([128, B * U1], BF16, tag=f"decT_{k}", name=f"decT_{k}")
                    for k in range(4)]

            mbuf = pp.tile([8, 8], F32, tag="mbuf", name="mbuf")
            nc.vector.memset(mbuf[:, :], 1.0)

            # ============ phase 0: xp0 batch + enc ============
            with tc.tile_pool(name="prep", bufs=1) as prp, \
                 tc.tile_pool(name="prep_ps", bufs=1, space="PSUM") as prps:
                hsT = []
                for k in range(4):
                    t = prp.tile([128, NTB], BF16, tag=f"hsT_{k}", name=f"hsT_{k}")
                    nc.sync.dma_start(t[:, :], hsT_d.ap()[128 * k:128 * (k + 1), :])
                    hsT.append(t)
                lencT = []
                for k in range(4):
                    t = prp.tile([128, D], BF16, tag=f"lencT_{k}", name=f"lencT_{k}")
                    nc.sync.dma_start(t[:, :], lencT_d.ap()[128 * k:128 * (k + 1), :])
                    lencT.append(t)
                eysT = []
                for k in range(4):
                    t = prp.tile([128, U1 * B], BF16, tag=f"eysT_{k}", name=f"eysT_{k}")
                    nc.sync.dma_start(t[:, :], eysT_d.ap()[128 * k:128 * (k + 1), :])
                    eysT.append(t)
                wih0 = []
                for k in range(4):
                    t = prp.tile([128, G], BF16, tag=f"wih0_{k}", name=f"wih0_{k}")
                    nc.sync.dma_start(t[:, :], wih0T_d.ap()[128 * k:128 * (k + 1), :])
                    wih0.append(t)
                bias0 = prp.tile([1, G], BF16, tag="bias0", name="bias0")
                nc.sync.dma_start(bias0[:, :], bias0_d.ap())

                m_sizes = [128, 128, 128, 24]
                for m in range(4):
                    mo, mr = 128 * m, m_sizes[m]
                    nu = mr // 8
                    u0 = mo // 8
                    ps = prps.tile([128, G], F32, tag="xp_ps", name="xp_ps")
                    for k in range(4):
                        for n in range(4):
                            nc.tensor.matmul(
                                ps[0:mr, 512 * n:512 * (n + 1)],
                                lhsT=eysT[k][:, mo:mo + mr],
                                rhs=wih0[k][:, 512 * n:512 * (n + 1)],
                                start=(k == 0), stop=False)
                    for n in range(4):
                        nc.tensor.matmul(
                            ps[0:mr, 512 * n:512 * (n + 1)],
                            lhsT=onesb[0:1, 0:mr],
                            rhs=bias0[0:1, 512 * n:512 * (n + 1)],
                            start=False, stop=True)
                    stg = prp.tile([128, G], F32, tag="xp_stg", name="xp_stg")
                    nc.vector.tensor_copy(stg[0:mr, :], ps[0:mr, :])
                    # banded store: xp0_i[u, 32n+b, :] = xp[b, u, n-chunk]
                    for n in range(4):
                        nc.sync.dma_start(
                            xp_i[0].ap()[u0:u0 + nu, 32 * n:32 * n + 8, :],
                            stg[0:mr, 512 * n:512 * (n + 1)]
                                .rearrange("(u b) j -> u b j", b=8))

                # enc = hs @ lenc.T + lencb  (per j-chunk); cols (b, t)
                for jc in range(4):
                    eps = prps.tile([128, NTB], F32, tag="enc_ps", name="enc_ps")
                    for k in range(4):
                        nc.tensor.matmul(eps[:, :],
                                         lhsT=lencT[k][:, 128 * jc:128 * (jc + 1)],
                                         rhs=hsT[k][:, :], start=(k == 0),
                                         stop=(k == 3))
                    nc.vector.tensor_scalar(out=encT[jc][:, :], in0=eps[:, :],
                                            scalar1=lencb[:, jc:jc + 1],
                                            scalar2=None, op0=OP.add)

            # ============ LSTM recurrence ============
            with tc.tile_pool(name="lstmw", bufs=1) as lwp, \
                 tc.tile_pool(name="step", bufs=3) as sp, \
                 tc.tile_pool(name="lstm_ps", bufs=2, space="PSUM") as lps, \
                 tc.tile_pool(name="tp_ps", bufs=2, space="PSUM") as tps, \
                 tc.tile_pool(name="xp1_ps", bufs=1, space="PSUM") as xps:

                whh = {}
                for l, wd in ((0, whh0T_d), (1, whh1T_d)):
                    whh[l] = []
                    for k in range(4):
                        t = lwp.tile([128, G], BF16, tag=f"whh{l}_{k}",
                                     name=f"whh{l}_{k}")
                        nc.sync.dma_start(t[:, :], wd.ap()[128 * k:128 * (k + 1), :])
                        whh[l].append(t)
                wih1 = []
                for k in range(4):
                    t = lwp.tile([128, G], BF16, tag=f"wih1_{k}", name=f"wih1_{k}")
                    nc.sync.dma_start(t[:, :], wih1T_d.ap()[128 * k:128 * (k + 1), :])
                    wih1.append(t)
                bias1 = lwp.tile([1, G], BF16, tag="bias1", name="bias1")
                nc.sync.dma_start(bias1[:, :], bias1_d.ap())

                # c-state per layer at partitions 32..40
                CST = {l: lwp.tile([40, D], F32, tag=f"c{l}", name=f"c{l}")
                       for l in (0, 1)}

                def lstm_step(l, u):
                    # xp for this step, banded (g@0, i@32, o@64, f@96)
                    xpt = sp.tile([128, D], F32, tag=f"xpt{l}", name=f"xpt{l}")
                    nc.sync.dma_start(xpt[0:104, :], xp_i[l].ap()[u])
                    gp = lps.tile([128, D], F32, tag=f"gp{l}", name=f"gp{l}")
                    if u > 0:
                        for k in range(4):
                            lhsT = hT[l][:, (k * U1 + u - 1) * 8:(k * U1 + u) * 8]
                            for n in range(4):
                                nc.tensor.matmul(
                                    gp[32 * n:32 * n + 8, :], lhsT=lhsT,
                                    rhs=whh[l][k][:, 512 * n:512 * (n + 1)],
                                    start=(k == 0), stop=(k == 3),
                                    tile_position=(0, 32 * n))
                        AS = sp.tile([128, D], F32, tag=f"as{l}", name=f"as{l}")
                        nc.vector.tensor_tensor(out=AS[0:104, :], in0=gp[0:104, :],
                                                in1=xpt[0:104, :], op=OP.add)
                    else:
                        AS = xpt
                    TGS = sp.tile([8, D], F32, tag=f"tg{l}", name=f"tg{l}")
                    nc.scalar.activation(TGS[:, :], AS[0:8, :], AF.Tanh)
                    # sigmoid of all bands written back into PSUM gp so the
                    # c-chain ops pair PSUM with SBUF (cross-base legal)
                    nc.scalar.activation(gp[0:104, :], AS[0:104, :], AF.Sigmoid)
                    c = CST[l]
                    X2 = sp.tile([40, D], F32, tag=f"x2{l}", name=f"x2{l}")
                    # c2 = tanh(g) * sigm(i)
                    nc.vector.tensor_tensor(out=X2[32:40, :], in0=TGS[:, :],
                                            in1=gp[32:40, :], op=OP.mult)
                    if u == 0:
                        nc.vector.tensor_copy(c[32:40, :], X2[32:40, :])
                    else:
                        X1 = sp.tile([40, D], F32, tag=f"x1{l}", name=f"x1{l}")
                        # c1 = sigm(f) * c
                        nc.vector.tensor_tensor(out=X1[32:40, :],
                                                in0=gp[96:104, :],
                                                in1=c[32:40, :], op=OP.mult)
                        nc.gpsimd.tensor_tensor(out=c[32:40, :],
                                                in0=X1[32:40, :],
                                                in1=X2[32:40, :], op=OP.add)
                    TH = sp.tile([40, D], F32, tag=f"th{l}", name=f"th{l}")
                    nc.scalar.activation(TH[32:40, :], c[32:40, :], AF.Tanh)
                    HB = sp.tile([8, D], BF16, tag=f"hb{l}", name=f"hb{l}")
                    # h = sigm(o) * tanh(c) (psum band64 x sbuf band32)
                    nc.vector.tensor_tensor(out=HB[0:8, :], in0=gp[64:72, :],
                                            in1=TH[32:40, :], op=OP.mult)
                    tp = tps.tile([128, 32], BF16, tag="tp", name="tp")
                    for k in range(4):
                        nc.tensor.transpose(tp[:, 8 * k:8 * (k + 1)],
                                            HB[:, 128 * k:128 * (k + 1)],
                                            ident8[:, :])
                    nc.scalar.copy(
                        hT[l][:, :].rearrange("p (k u b) -> p k u b",
                                              k=4, u=U1)[:, :, u, :],
                        tp[:, :].rearrange("p (k b) -> p k b", k=4))

                def xp1_block(bi):
                    u0, u1b = UBLOCKS[bi]
                    nu = u1b - u0
                    mr = 8 * nu
                    for h in range(2):
                        ps = xps.tile([128, 1024], F32, tag="xp1_ps",
                                      name="xp1_ps")
                        for k in range(4):
                            lhsT = hT[0][:, (k * U1 + u0) * 8:(k * U1 + u1b) * 8]
                            for n2 in range(2):
                                n = 2 * h + n2
                                nc.tensor.matmul(
                                    ps[0:mr, 512 * n2:512 * (n2 + 1)], lhsT=lhsT,
                                    rhs=wih1[k][:, 512 * n:512 * (n + 1)],
                                    start=(k == 0), stop=False)
                        for n2 in range(2):
                            n = 2 * h + n2
                            nc.tensor.matmul(
                                ps[0:mr, 512 * n2:512 * (n2 + 1)],
                                lhsT=onesb[0:1, 0:mr],
                                rhs=bias1[0:1, 512 * n:512 * (n + 1)],
                                start=False, stop=True)
                        stg1 = sp.tile([128, 1024], F32, tag="xp1_stg",
                                       name="xp1_stg")
                        nc.vector.tensor_copy(stg1[0:mr, :], ps[0:mr, :])
                        for n2 in range(2):
                            n = 2 * h + n2
                            nc.sync.dma_start(
                                xp_i[1].ap()[u0:u0 + nu, 32 * n:32 * n + 8, :],
                                stg1[0:mr, 512 * n2:512 * (n2 + 1)]
                                    .rearrange("(u b) j -> u b j", b=8))

                block_end = {7: 0, 15: 1, 23: 2, 31: 3, 39: 4, 47: 5, 50: 6}
                for up in range(U1 + SHIFT):
                    if up < U1:
                        lstm_step(0, up)
                    if 0 <= up - SHIFT < U1:
                        lstm_step(1, up - SHIFT)
                    if up in block_end:
                        xp1_block(block_end[up])

            # ============ dec projection ============
            with tc.tile_pool(name="dec_ps", bufs=2, space="PSUM") as dps:
                for jc in range(4):
                    ps = dps.tile([128, U1 * B], F32, tag="dec_ps", name="dec_ps")
                    for k in range(4):
                        nc.tensor.matmul(
                            ps[:, :], lhsT=ldecT[k][:, 128 * jc:128 * (jc + 1)],
                            rhs=hT[1][:, k * U1 * 8:(k + 1) * U1 * 8],
                            start=(k == 0), stop=(k == 3))
                    # reorder columns (u*8+b) -> (b*51+u)
                    nc.vector.tensor_copy(
                        decT[jc][:, :].rearrange("p (b u) -> p b u", b=B),
                        ps[:, :].rearrange("p (u b) -> p b u", u=U1))

            # ============ joint ============
            with tc.tile_pool(name="joint", bufs=2) as jp, \
                 tc.tile_pool(name="jexp", bufs=3) as jep, \
                 tc.tile_pool(name="z1_pool", bufs=1) as z1p, \
                 tc.tile_pool(name="joint_ps", bufs=4, space="PSUM") as jps:

                z1T = [z1p.tile([128, RPAD], BF16, tag=f"z1T_{k}",
                                name=f"z1T_{k}") for k in range(4)]
                # z1 = tanh(enc + dec); rows ordered (b, t, u)
                for jc in range(4):
                    for b in range(B):
                        zs = jp.tile([128, TSH * U1], BF16, tag="zs", name="zs")
                        in0 = encT[jc][:, TSH * b:TSH * (b + 1)] \
                            .unsqueeze(2).broadcast_to([128, TSH, U1])
                        in1 = decT[jc][:, U1 * b:U1 * (b + 1)] \
                            .unsqueeze(1).broadcast_to([128, TSH, U1])
                        nc.gpsimd.tensor_tensor(
                            out=zs[:, :].rearrange("p (t u) -> p t u", t=TSH),
                            in0=in0, in1=in1, op=OP.add)
                        nc.scalar.activation(
                            z1T[jc][:, 1275 * b:1275 * (b + 1)], zs[:, :],
                            AF.Tanh)
                for jc in range(4):
                    nc.vector.memset(z1T[jc][:, ROWS:RPAD], 0.0)

                rs_all = jp.tile([128, NMT], F32, tag="rs_all", name="rs_all")
                eb_all = jp.tile([128, NMT], F32, tag="eb_all", name="eb_all")
                el_all = jp.tile([128, NMT], F32, tag="el_all", name="el_all")

                for m in range(NMT):
                    mo = 128 * m
                    zp = jps.tile([128, 512], F32, tag="zp", name="zp")
                    for k in range(4):
                        nc.tensor.matmul(zp[:, :], lhsT=z1T[k][:, mo:mo + 128],
                                         rhs=loutT[k][:, :], start=(k == 0),
                                         stop=False)
                    nc.tensor.matmul(zp[:, :], lhsT=onesb[0:1, :],
                                     rhs=loutb[0:1, :], start=False, stop=True)
                    ez = jep.tile([128, 512], F32, tag="ez", name="ez")
                    nc.scalar.activation(ez[:, :], zp[:, :], AF.Exp,
                                         accum_out=rs_all[:, m:m + 1])
                    nc.vector.tensor_copy(eb_all[:, m:m + 1], ez[:, 0:1])
                    scr = jep.tile([128, 512], F32, tag="scr", name="scr")
                    nc.vector.scalar_tensor_tensor(
                        out=scr[:, :], in0=iot[:, :],
                        scalar=ysidx[:, m:m + 1], in1=ez[:, :],
                        op0=OP.is_equal, op1=OP.mult,
                        accum_out=el_all[:, m:m + 1])

                # pb = eb * EC / rs ; pl = el * EC / rs  (bf16)
                rec = jp.tile([128, NMT], F32, tag="rec", name="rec")
                nc.vector.reciprocal(rec[:, :], rs_all[:, :])
                nc.vector.tensor_scalar(out=rec[:, :], in0=rec[:, :], scalar1=EC,
                                        scalar2=None, op0=OP.mult)
                ebb = jp.tile([128, 128], BF16, tag="ebb", name="ebb")
                elb = jp.tile([128, 128], BF16, tag="elb", name="elb")
                nc.vector.memset(ebb[:, NMT:128], 0.0)
                nc.vector.memset(elb[:, NMT:128], 0.0)
                nc.vector.tensor_tensor(out=ebb[:, 0:NMT], in0=eb_all[:, :],
                                        in1=rec[:, :], op=OP.mult)
                nc.vector.tensor_tensor(out=elb[:, 0:NMT], in0=el_all[:, :],
                                        in1=rec[:, :], op=OP.mult)
                # transpose to (m, p) row-major and store
                ebT = jp.tile([128, 128], BF16, tag="ebT", name="ebT")
                elT = jp.tile([128, 128], BF16, tag="elT", name="elT")
                nc.sync.dma_start_transpose(ebT[:, :], ebb[:, :])
                nc.sync.dma_start_transpose(elT[:, :], elb[:, :])
                nc.sync.dma_start(
                    lp_loc.ap()[0:RPAD].rearrange("(m p) -> m p", p=128),
                    ebT[0:NMT, :])
                nc.sync.dma_start(
                    lp_loc.ap()[RPAD:2 * RPAD].rearrange("(m p) -> m p", p=128),
                    elT[0:NMT, :])

            # ============ all-gather ============
            nc.gpsimd.collective_compute(
                "AllGather", OP.bypass, replica_groups=[list(range(NCORES))],
                ins=[lp_loc.ap()], outs=[lp_full.ap()])

            # ============ DP ============
            with tc.tile_pool(name="dp", bufs=1) as dp:
                pb = dp.tile([8, T * U1], BF16, tag="pb", name="pb")
                pl = dp.tile([8, T * U1], BF16, tag="pl", name="pl")
                for r in range(NCORES):
                    base = r * 2 * RPAD
                    for (dst, off) in ((pb, 0), (pl, RPAD)):
                        nc.sync.dma_start(
                            dst[:, 1275 * r:1275 * (r + 1)],
                            lp_full.ap()[base + off:base + off + ROWS]
                                .rearrange("(b x) -> b x", b=8))

                A = [dp.tile([8, U1], F32, tag=f"A{i}", name=f"A{i}")
                     for i in (0, 1)]
                zz = dp.tile([8, U], F32, tag="zz", name="zz")
                nc.vector.memset(zz[:, :], 0.0)
                nc.vector.memset(A[0][:, 0:1], 1.0)
                nc.vector.tensor_tensor_scan(
                    out=A[0][:, 1:U1], data0=pl[:, 0:U], data1=zz[:, :],
                    initial=1.0, op0=OP.mult, op1=OP.add)
                nren = 0
                for t in range(1, T):
                    cur, nxt = A[(t - 1) % 2], A[t % 2]
                    nc.vector.tensor_tensor(
                        out=nxt[:, :], in0=cur[:, :],
                        in1=pb[:, U1 * (t - 1):U1 * t], op=OP.mult)
                    nc.vector.tensor_tensor_scan(
                        out=nxt[:, 1:U1], data0=pl[:, U1 * t:U1 * t + U],
                        data1=nxt[:, 1:U1], initial=nxt[:, 0:1],
                        op0=OP.mult, op1=OP.add)
                    if t % RENORM_EVERY == 0 and t < T - 1:
                        mx = dp.tile([8, 1], F32, tag="mx", name="mx")
                        nc.vector.tensor_reduce(out=mx[:, :], in_=nxt[:, :],
                                                axis=mybir.AxisListType.X,
                                                op=OP.max)
                        nc.vector.tensor_copy(mbuf[:, nren:nren + 1], mx[:, :])
                        rcp = dp.tile([8, 1], F32, tag="rcp", name="rcp")
                        nc.vector.reciprocal(rcp[:, :], mx[:, :])
                        nc.vector.tensor_scalar(out=nxt[:, :], in0=nxt[:, :],
                                                scalar1=rcp[:, 0:1],
                                                scalar2=None, op0=OP.mult)
                        nren += 1

                fin = A[(T - 1) % 2]
                nc.vector.tensor_tensor(
                    out=mbuf[:, 7:8], in0=fin[:, U:U1],
                    in1=pb[:, U1 * (T - 1) + U:U1 * (T - 1) + U1], op=OP.mult)
                lg = dp.tile([8, 8], F32, tag="lg", name="lg")
                nc.scalar.activation(lg[:, :], mbuf[:, :], AF.Ln)
                ssum = dp.tile([8, 1], F32, tag="ssum", name="ssum")
                nc.vector.tensor_reduce(out=ssum[:, :], in_=lg[:, :],
                                        axis=mybir.AxisListType.X, op=OP.add)
                tt = dp.tile([32, 32], F32, tag="tt", name="tt")
                nc.vector.memset(tt[:, :], 0.0)
                nc.vector.tensor_copy(tt[0:8, 0:1], ssum[:, :])
                tu = dp.tile([32, 32], F32, tag="tu", name="tu")
                nc.vector.transpose(tu[:, :], tt[:, :])
                tot = dp.tile([1, 1], F32, tag="tot", name="tot")
                nc.vector.tensor_reduce(out=tot[:, :], in_=tu[0:1, :],
                                        axis=mybir.AxisListType.X, op=OP.add)
                nc.vector.tensor_scalar(out=tot[:, :], in0=tot[:, :],
                                        scalar1=-0.125, scalar2=250.0 * C_SHIFT,
                                        op0=OP.mult, op1=OP.add)
                nc.sync.dma_start(loss_d.ap(), tot[:, :])

    nc.compile()
    return nc


def _to_bf(x):
    return np.ascontiguousarray(x).astype(BF)


def _host_prep(inputs):
    hs_pad = np.asarray(inputs["hs_pad"], np.float32)
    ys_pad = np.asarray(inputs["ys_pad"])
    embed_w = np.asarray(inputs["embed_w"], np.float32)

    Emb = embed_w.copy()
    Emb[0] = 0.0
    ys_in = np.concatenate([np.zeros((B, 1), ys_pad.dtype), ys_pad], axis=1)
    eys = Emb[ys_in]                                   # (B, U1, D)
    eysT = np.ascontiguousarray(eys.transpose(2, 1, 0)).reshape(D, U1 * B)

    def wT(w):
        return np.ascontiguousarray(np.asarray(w, np.float32)[PERM].T)

    common = {
        "eysT": _to_bf(eysT),
        "wih0T": _to_bf(wT(inputs["w_ih0"])),
        "whh0T": _to_bf(wT(inputs["w_hh0"])),
        "wih1T": _to_bf(wT(inputs["w_ih1"])),
        "whh1T": _to_bf(wT(inputs["w_hh1"])),
        "bias0": _to_bf((np.asarray(inputs["b_ih0"], np.float32)
                         + np.asarray(inputs["b_hh0"], np.float32))[PERM][None, :]),
        "bias1": _to_bf((np.asarray(inputs["b_ih1"], np.float32)
                         + np.asarray(inputs["b_hh1"], np.float32))[PERM][None, :]),
        "lencT": _to_bf(np.asarray(inputs["lin_enc_w"], np.float32).T),
        "ldecT": _to_bf(np.asarray(inputs["lin_dec_w"], np.float32).T),
        "loutT": _to_bf(np.asarray(inputs["lin_out_w"], np.float32).T),
        "lencb": np.ascontiguousarray(
            np.asarray(inputs["lin_enc_b"], np.float32).reshape(4, 128).T),
        "loutb": _to_bf(np.asarray(inputs["lin_out_b"], np.float32)[None, :]),
        "ident8": _to_bf(np.eye(8, dtype=np.float32)),
    }

    # label index per padded row r = (b*25+tl)*51 + u  (same for every core)
    r = np.arange(RPAD)
    bb = r // (TSH * U1)
    uu = r % U1
    vals = np.where((uu < U) & (r < ROWS),
                    ys_pad[np.minimum(bb, B - 1), np.minimum(uu, U - 1)], -1.0)
    common["ysidx"] = np.ascontiguousarray(
        vals.reshape(NMT, 128).T.astype(np.float32))

    in_maps = []
    for c in range(NCORES):
        hs = hs_pad[:, TSH * c:TSH * (c + 1), :]       # (B, 25, D)
        hsT = np.ascontiguousarray(hs.transpose(2, 0, 1)).reshape(D, NTB)
        m = dict(common)
        m["hsT"] = _to_bf(hsT)
        in_maps.append(m)
    return in_maps


def kernel(**inputs):
    if "nc" not in _BUILD_CACHE:
        _BUILD_CACHE["nc"] = build()
    nc = _BUILD_CACHE["nc"]
    in_maps = _host_prep(inputs)
    res = bass_utils.run_bass_kernel_spmd(nc, in_maps,
                                          core_ids=list(range(NCORES)))
    return np.float32(res.results[0]["loss"][0, 0])


if __name__ == "__main__":
    dat = np.load("/root/problem/inputs.npz")
    out = kernel(**{k: dat[k] for k in dat.files})
    exp = float(np.load("/root/problem/expected.npy"))
    rel = abs(float(out) - exp) / abs(exp)
    print("loss =", out, "expected =", exp, "rel err =", rel)

